# revision 1
# baseline (speedup 1.0000x reference)
"""Causal self-attention Trainium2 Bass kernel.

Problem: B=4, T=2048, C=1024, H=16 heads, Dh=64, causal, f32 I/O.

Sharding (8 NeuronCores): 4 batches x 2 head-groups. Core (b, g) handles
batch b and heads [8g, 8g+8). The qkv projection is column-sharded over
heads, the output projection row-sharded; each core emits a partial
y [2048, 1024] and the host sums the two partials per batch. No
cross-device communication.

Per-core algorithm (all matmuls bf16, f32 PSUM accumulate):
  - inputs arrive pre-transposed + bf16-cast from the host:
      xT [C, T], wqkT [C, 1024] (q|k features), wvT [C, 512], woT [512, C]
  - qkT [o, t] = wqkT.T-style matmuls (contraction C on partitions)
  - v [t, o] per head packed into v_aug [128, 16, 65] with a ones column
    (col 64) so the P@V matmul also produces the softmax denominator l
  - attention per head-pair, S^T layout [key, query]:
      ST = kT.T-slices @ qT-slices, two heads packed on the PE array via
      tile_position (0,0)/(64,0) row-tiling (contraction is Dh=64)
      P = exp(0.125 * ST) (scale folded into the ACT exp); block-causal
      masks multiplied in for diagonal chunks (gpsimd)
      oT[d, i] (+ l at row 64) = v_aug.T @ P accumulated over key chunks
      attT = oT * bcast(1/l), where the per-query 1/l row is broadcast
      across partitions with a tiny K=2 matmul (bf16 hi/lo split)
  - y [t, o] = attT.T-slices @ woT, accumulated over head pairs
"""

import os
import sys

sys.path.insert(0, "/opt/trn_rl_repo")

from contextlib import ExitStack

import ml_dtypes
import numpy as np

import concourse.mybir as mybir
import concourse.tile as tile
from concourse import bacc
from concourse.bass_utils import run_bass_kernel_spmd

F32 = mybir.dt.float32
BF16 = mybir.dt.bfloat16
BF = ml_dtypes.bfloat16
EXP = mybir.ActivationFunctionType.Exp
MUL = mybir.AluOpType.mult
SUB = mybir.AluOpType.subtract

P = 128
T = 2048
C = 1024
NT128 = T // 128  # 16
NT512 = T // 512  # 4
CC = C // P  # 8 contraction chunks
SCALE = 0.125  # 1/sqrt(64)

LAST_RESULTS = None  # BassKernelResults of the most recent run (for test.py)


def _build():
    nc = bacc.Bacc(trn_type="TRN2")

    xT_d = nc.dram_tensor("xT", [C, T], BF16, kind="ExternalInput")
    wqkT_d = nc.dram_tensor("wqkT", [C, 1024], BF16, kind="ExternalInput")
    wvT_d = nc.dram_tensor("wvT", [C, 512], BF16, kind="ExternalInput")
    woT_d = nc.dram_tensor("woT", [512, C], BF16, kind="ExternalInput")
    out_d = nc.dram_tensor("out", [T, C], F32, kind="ExternalOutput")

    # Block-causal masks for the 4 diagonal key-chunks of each 512-query
    # i-block: mask[r][j, i] = 1 iff query i >= key j + 128 r.
    ii = np.arange(512)[None, :]
    jj = np.arange(128)[:, None]
    masks_np = np.stack([(ii >= jj + 128 * r) for r in range(4)]).astype(BF)
    masks_d = nc.inline_tensor(masks_np, name="masks")

    # Indicator for the recip broadcast matmul: bc rows 0:64 take rhs row 0
    # (recip of head A's l), rows 64:128 take rhs row 64 (head B). Rows
    # 1..63 and 65.. of the rhs are junk (memset to 1.0) and are zeroed by
    # the indicator columns.
    ind_np = np.zeros((65, 128), BF)
    ind_np[0, :64] = 1
    ind_np[64, 64:] = 1
    ind_d = nc.inline_tensor(ind_np, name="ind")

    with tile.TileContext(nc) as tc, ExitStack() as ctx:
        persist = ctx.enter_context(tc.tile_pool(name="persist", bufs=1))
        ppool = ctx.enter_context(tc.tile_pool(name="ppool", bufs=3))
        sbm = ctx.enter_context(tc.tile_pool(name="sbm", bufs=3))
        ysb = ctx.enter_context(tc.tile_pool(name="ysb", bufs=3))
        pst = ctx.enter_context(tc.tile_pool(name="pst", bufs=2, space="PSUM"))
        pot = ctx.enter_context(tc.tile_pool(name="pot", bufs=2, space="PSUM"))
        pmisc = ctx.enter_context(tc.tile_pool(name="pmisc", bufs=2, space="PSUM"))

        # ---------------- persistent loads ----------------
        maskt = []
        for r in range(4):
            m = persist.tile([P, 512], BF16, tag=f"mask{r}")
            nc.sync.dma_start(m, masks_d[r, :, :])
            maskt.append(m)
        ind_sb = persist.tile([65, P], BF16, tag="ind")
        nc.sync.dma_start(ind_sb, ind_d[:, :])

        wqk = []
        wv = []
        for c in range(CC):
            w1 = persist.tile([P, 1024], BF16, tag=f"wqk{c}")
            nc.sync.dma_start(w1, wqkT_d[c * P : (c + 1) * P, :])
            wqk.append(w1)
            w2 = persist.tile([P, 512], BF16, tag=f"wv{c}")
            nc.sync.dma_start(w2, wvT_d[c * P : (c + 1) * P, :])
            wv.append(w2)
        wo = []
        for p4 in range(4):
            w3 = persist.tile([P, 1024], BF16, tag=f"wo{p4}")
            nc.sync.dma_start(w3, woT_d[p4 * P : (p4 + 1) * P, :])
            wo.append(w3)
        xk = []
        for c in range(CC):
            xt = persist.tile([P, T], BF16, tag=f"xT{c}")
            nc.sync.dma_start(xt, xT_d[c * P : (c + 1) * P, :])
            xk.append(xt)

        # v_aug[h]: [128 t, 16 j-chunk, 65] with ones in column 64
        vaug = []
        for h in range(8):
            va = persist.tile([P, NT128, 65], BF16, tag=f"vaug{h}")
            nc.vector.memset(va[:, :, 64:65], 1.0)
            vaug.append(va)

        # attT[p]: [128 = 2 heads x 64 d, T] bf16
        attT = [
            persist.tile([P, T], BF16, tag=f"attT{p}", name=f"attT{p}")
            for p in range(4)
        ]

        # ---------------- v projection ----------------
        for t in range(NT128):
            pv = pmisc.tile([P, 512], F32, tag="m")
            for c in range(CC):
                nc.tensor.matmul(
                    pv,
                    xk[c][:, t * P : (t + 1) * P],
                    wv[c],
                    start=(c == 0),
                    stop=(c == CC - 1),
                )
            for h in range(8):
                nc.vector.tensor_copy(
                    vaug[h][:, t, 0:64], pv[:, h * 64 : (h + 1) * 64]
                )

        # ---------------- qk projection ----------------
        # o-chunk o covers features [128 o, 128 o + 128); chunks 0-3 are q,
        # 4-7 are k. Emit in pair order so attention can start early.
        qkT = {}
        for o in (0, 4, 1, 5, 2, 6, 3, 7):
            qo = persist.tile([P, T], BF16, tag=f"qkT{o}")
            qkT[o] = qo
            for t4 in range(NT512):
                pq = pmisc.tile([P, 512], F32, tag="m")
                for c in range(CC):
                    nc.tensor.matmul(
                        pq,
                        wqk[c][:, o * P : (o + 1) * P],
                        xk[c][:, t4 * 512 : (t4 + 1) * 512],
                        start=(c == 0),
                        stop=(c == CC - 1),
                    )
                nc.vector.tensor_copy(qo[:, t4 * 512 : (t4 + 1) * 512], pq)

        # ---------------- attention + output projection ----------------
        for bi in range(NT512):
            i_sl = slice(bi * 512, (bi + 1) * 512)
            for p in range(4):
                qt = qkT[p]
                kt = qkT[4 + p]
                oA = pot.tile([65, 512], F32, tag="ot")
                oB = pot.tile([65, 512], F32, tag="ot")
                nj = 4 * bi + 4
                for bj in range(nj):
                    j_sl = slice(bj * P, (bj + 1) * P)
                    st = pst.tile([P, 1024], F32, tag="st")
                    nc.tensor.matmul(
                        st[:, 0:512], kt[0:64, j_sl], qt[0:64, i_sl],
                        start=True, stop=True, tile_position=(0, 0),
                    )
                    nc.tensor.matmul(
                        st[:, 512:1024], kt[64:128, j_sl], qt[64:128, i_sl],
                        start=True, stop=True, tile_position=(64, 0),
                    )
                    pt = ppool.tile([P, 1024], BF16, tag="pt")
                    nc.scalar.activation(pt, st, EXP, scale=SCALE)
                    if bj >= 4 * bi:
                        r = bj - 4 * bi
                        nc.vector.tensor_tensor(
                            pt[:, 0:512], pt[:, 0:512], maskt[r], MUL
                        )
                        nc.vector.tensor_tensor(
                            pt[:, 512:1024], pt[:, 512:1024], maskt[r], MUL
                        )
                    nc.tensor.matmul(
                        oA, vaug[2 * p][:, bj, :], pt[:, 0:512],
                        start=(bj == 0), stop=(bj == nj - 1),
                    )
                    nc.tensor.matmul(
                        oB, vaug[2 * p + 1][:, bj, :], pt[:, 512:1024],
                        start=(bj == 0), stop=(bj == nj - 1),
                    )
                # Short psum-release tail: copy unnormalized oT + l rows out
                # so the next pair's PE matmuls aren't gated on the recip
                # chain (keeps the HAM clock-gate warm).
                rc = sbm.tile([65, 512], F32, tag="rc")
                nc.vector.memset(rc, 1.0)
                nc.vector.tensor_copy(rc[0:1, :], oA[64:65, :])
                nc.vector.tensor_copy(rc[64:65, :], oB[64:65, :])
                nc.vector.tensor_copy(attT[p][0:64, i_sl], oA[0:64, :])
                nc.vector.tensor_copy(attT[p][64:128, i_sl], oB[0:64, :])
                # off-path: recip, hi/lo split, broadcast, in-place normalize
                nc.vector.reciprocal(rc, rc)
                hi = sbm.tile([65, 512], BF16, tag="hi")
                lof = sbm.tile([65, 512], F32, tag="lof")
                lo = sbm.tile([65, 512], BF16, tag="lo")
                nc.vector.tensor_copy(hi, rc)
                nc.vector.tensor_tensor(lof, rc, hi, SUB)
                nc.vector.tensor_copy(lo, lof)
                bcp = pmisc.tile([P, 512], F32, tag="m")
                nc.tensor.matmul(bcp, ind_sb, hi, start=True, stop=False)
                nc.tensor.matmul(bcp, ind_sb, lo, start=False, stop=True)
                bcs = sbm.tile([P, 512], F32, tag="bcs")
                nc.vector.tensor_copy(bcs, bcp)
                nc.vector.tensor_tensor(
                    attT[p][0:64, i_sl], attT[p][0:64, i_sl], bcs[0:64, :], MUL
                )
                nc.vector.tensor_tensor(
                    attT[p][64:128, i_sl], attT[p][64:128, i_sl],
                    bcs[64:128, :], MUL
                )
            # output projection for this i-block's t rows
            for t in range(4 * bi, 4 * bi + 4):
                y = ysb.tile([P, 1024], F32, tag="y")
                for oh in range(2):
                    py = pmisc.tile([P, 512], F32, tag="m")
                    for p in range(4):
                        nc.tensor.matmul(
                            py,
                            attT[p][:, t * P : (t + 1) * P],
                            wo[p][:, oh * 512 : (oh + 1) * 512],
                            start=(p == 0),
                            stop=(p == 3),
                        )
                    nc.vector.tensor_copy(y[:, oh * 512 : (oh + 1) * 512], py)
                nc.sync.dma_start(out_d[t * P : (t + 1) * P, :], y)

    nc.compile()
    return nc


_NC_CACHE = None


def _get_nc():
    global _NC_CACHE
    if _NC_CACHE is None:
        _NC_CACHE = _build()
    return _NC_CACHE


def kernel(x, W_qkv, W_out):
    global LAST_RESULTS
    x = np.asarray(x)
    W_qkv = np.asarray(W_qkv)
    W_out = np.asarray(W_out)
    B = x.shape[0]

    in_maps = []
    for b in range(B):
        xT = np.ascontiguousarray(x[b].T).astype(BF)
        for g in range(2):
            lo_, hi_ = 512 * g, 512 * g + 512
            wqkT = np.ascontiguousarray(
                np.concatenate([W_qkv[lo_:hi_], W_qkv[1024 + lo_ : 1024 + hi_]], 0).T
            ).astype(BF)
            wvT = np.ascontiguousarray(W_qkv[2048 + lo_ : 2048 + hi_].T).astype(BF)
            woT = np.ascontiguousarray(W_out[:, lo_:hi_].T).astype(BF)
            in_maps.append({"xT": xT, "wqkT": wqkT, "wvT": wvT, "woT": woT})

    nc = _get_nc()
    res = run_bass_kernel_spmd(nc, in_maps, core_ids=list(range(8)))
    LAST_RESULTS = res

    y = np.empty((B, T, C), np.float32)
    for b in range(B):
        y[b] = res.results[2 * b]["out"] + res.results[2 * b + 1]["out"]
    return y



# revision 2
# speedup vs baseline: 2.5113x; 2.5113x over previous
"""Causal self-attention Trainium2 Bass kernel.

Problem: B=4, T=2048, C=1024, H=16 heads, Dh=64, causal, f32 I/O.

Sharding (4 NeuronCores): pure data parallel on batch. Core b handles
batch b with all 16 heads, so x is never replicated across cores and
each core emits its complete y [2048, 1024] slice — no partial sums, no
cross-device communication.

The projection weights are module parameters, so they are baked into the
NEFF as Const tensors (DMA'd to HBM once at model load) instead of being
streamed in as per-exec inputs; the build is cached on a hash of the
weight bytes and transparently redone if kernel() is ever called with
different weights. The only per-exec input is xT [C, T] bf16; the only
output is y [T, C] f32.

Per-core algorithm (all matmuls bf16, f32 PSUM accumulate):
  - xT arrives pre-transposed + bf16-cast from the host
  - v [t, f] per head packed into v_aug [128, 16, 65] with a ones column
    (col 64) so the P@V matmul also produces the softmax denominator l
  - per head-pair p (heads 2p, 2p+1):
      qT/kT [128, T] = W-slices.T @ x-slices (contraction C on partitions)
      attention in S^T layout [key, query]:
        ST = kT.T-slices @ qT-slices, two heads packed on the PE array via
        tile_position (0,0)/(64,0) row-tiling (contraction is Dh=64)
        P = exp(0.125 * ST) (scale folded into the ACT exp); block-causal
        masks multiplied in for diagonal chunks
        oT[d, i] (+ l at row 64) = v_aug.T @ P accumulated over key chunks
        attT = oT * bcast(1/l), where the per-query 1/l row is broadcast
        across partitions with a tiny K=2 matmul (bf16 hi/lo split)
  - y [t, o] = attT.T-slices @ woT, accumulated over all 8 pairs
"""

import hashlib
import sys

sys.path.insert(0, "/opt/trn_rl_repo")

from contextlib import ExitStack

import ml_dtypes
import numpy as np

import concourse.mybir as mybir
import concourse.tile as tile
from concourse import bacc
from concourse.bass_utils import run_bass_kernel_spmd

F32 = mybir.dt.float32
BF16 = mybir.dt.bfloat16
BF = ml_dtypes.bfloat16
EXP = mybir.ActivationFunctionType.Exp
MUL = mybir.AluOpType.mult
SUB = mybir.AluOpType.subtract

P = 128
T = 2048
C = 1024
NT128 = T // 128  # 16
NT512 = T // 512  # 4
CC = C // P  # 8 contraction chunks
NP = 8  # head pairs per core (16 heads)
SCALE = 0.125  # 1/sqrt(64)

LAST_RESULTS = None  # BassKernelResults of the most recent run (for test.py)


def _prep_weights(W_qkv, W_out):
    # wqk [16, 8, 128, 128]: [o-chunk, c-chunk, c_lo, o_lo]; o-chunks 0-7
    # are q features for pair o, 8-15 are k for pair o-8. Contiguous 32KB
    # per (o, c) slice so the per-pair weight DMAs are linear.
    wqkT = np.ascontiguousarray(W_qkv[0:2048].T).astype(BF)  # [1024 c, 2048 o]
    wqk = np.ascontiguousarray(
        wqkT.reshape(CC, P, 16, P).transpose(2, 0, 1, 3)
    )
    wvT = np.ascontiguousarray(W_qkv[2048:3072].T).astype(BF)  # [1024 c, 1024 f]
    woT = np.ascontiguousarray(W_out.T).astype(BF)  # [1024 c, 1024 o]
    return wqk, wvT, woT


def _build(wqk_np, wvT_np, woT_np):
    nc = bacc.Bacc(trn_type="TRN2")

    xT_d = nc.dram_tensor("xT", [C, T], BF16, kind="ExternalInput")
    out_d = nc.dram_tensor("out", [T, C], F32, kind="ExternalOutput")

    wqk_d = nc.inline_tensor(wqk_np, name="wqk")
    wv_d = nc.inline_tensor(wvT_np, name="wv")
    wo_d = nc.inline_tensor(woT_np, name="wo")

    # Block-causal masks for the 4 diagonal key-chunks of each 512-query
    # i-block: mask[r][j, i] = 1 iff query i >= key j + 128 r.
    ii = np.arange(512)[None, :]
    jj = np.arange(128)[:, None]
    masks_np = np.stack([(ii >= jj + 128 * r) for r in range(4)]).astype(BF)
    masks_d = nc.inline_tensor(masks_np, name="masks")

    # Indicator for the recip broadcast matmul: bc rows 0:64 take rhs row 0
    # (recip of head A's l), rows 64:128 take rhs row 64 (head B). Rows
    # 1..63 and 65.. of the rhs are junk (memset to 1.0) and are zeroed by
    # the indicator columns.
    ind_np = np.zeros((65, 128), BF)
    ind_np[0, :64] = 1
    ind_np[64, 64:] = 1
    ind_d = nc.inline_tensor(ind_np, name="ind")

    with tile.TileContext(nc) as tc, ExitStack() as ctx:
        persist = ctx.enter_context(tc.tile_pool(name="persist", bufs=1))
        ppool = ctx.enter_context(tc.tile_pool(name="ppool", bufs=3))
        sbm = ctx.enter_context(tc.tile_pool(name="sbm", bufs=3))
        ysb = ctx.enter_context(tc.tile_pool(name="ysb", bufs=3))
        pst = ctx.enter_context(tc.tile_pool(name="pst", bufs=2, space="PSUM"))
        pot = ctx.enter_context(tc.tile_pool(name="pot", bufs=2, space="PSUM"))
        pmisc = ctx.enter_context(tc.tile_pool(name="pmisc", bufs=2, space="PSUM"))

        # ---------------- persistent loads ----------------
        maskt = []
        for r in range(4):
            m = persist.tile([P, 512], BF16, tag=f"mask{r}")
            nc.sync.dma_start(m, masks_d[r, :, :])
            maskt.append(m)
        ind_sb = persist.tile([65, P], BF16, tag="ind")
        nc.sync.dma_start(ind_sb, ind_d[:, :])

        # v_aug[h]: [128 t, 16 j-chunk, 65] with ones in column 64
        vaug = []
        for h in range(2 * NP):
            va = persist.tile([P, NT128, 65], BF16, tag=f"vaug{h}")
            nc.vector.memset(va[:, :, 64:65], 1.0)
            vaug.append(va)

        # attT[p]: [128 = 2 heads x 64 d, T] bf16
        attT = [
            persist.tile([P, T], BF16, tag=f"attT{p}", name=f"attT{p}")
            for p in range(NP)
        ]

        with tc.tile_pool(name="proj", bufs=1) as proj, tc.tile_pool(
            name="wqs", bufs=2
        ) as wqs, tc.tile_pool(name="qkp", bufs=2) as qkp:
            xk = []
            for c in range(CC):
                xt = proj.tile([P, T], BF16, tag=f"xT{c}")
                nc.sync.dma_start(xt, xT_d[c * P : (c + 1) * P, :])
                xk.append(xt)
            wv = []
            for c in range(CC):
                w2 = proj.tile([P, C], BF16, tag=f"wv{c}")
                nc.sync.dma_start(w2, wv_d[c * P : (c + 1) * P, :])
                wv.append(w2)

            # ---------------- v projection ----------------
            for t in range(NT128):
                for half in range(2):
                    pv = pmisc.tile([P, 512], F32, tag="m")
                    for c in range(CC):
                        nc.tensor.matmul(
                            pv,
                            xk[c][:, t * P : (t + 1) * P],
                            wv[c][:, half * 512 : (half + 1) * 512],
                            start=(c == 0),
                            stop=(c == CC - 1),
                        )
                    for hh in range(8):
                        h = 8 * half + hh
                        nc.vector.tensor_copy(
                            vaug[h][:, t, 0:64], pv[:, hh * 64 : (hh + 1) * 64]
                        )

            # ------------- per-pair qk projection + attention -------------
            for p in range(NP):
                qt = qkp.tile([P, T], BF16, tag="qt")
                kt = qkp.tile([P, T], BF16, tag="kt")
                for which, dst in ((p, qt), (NP + p, kt)):
                    ws = []
                    for c in range(CC):
                        w1 = wqs.tile([P, P], BF16, tag=f"wq{c}")
                        nc.sync.dma_start(w1, wqk_d[which, c, :, :])
                        ws.append(w1)
                    for t4 in range(NT512):
                        pq = pmisc.tile([P, 512], F32, tag="m")
                        for c in range(CC):
                            nc.tensor.matmul(
                                pq,
                                ws[c],
                                xk[c][:, t4 * 512 : (t4 + 1) * 512],
                                start=(c == 0),
                                stop=(c == CC - 1),
                            )
                        nc.vector.tensor_copy(dst[:, t4 * 512 : (t4 + 1) * 512], pq)

                for bi in range(NT512):
                    i_sl = slice(bi * 512, (bi + 1) * 512)
                    oA = pot.tile([65, 512], F32, tag="ot")
                    oB = pot.tile([65, 512], F32, tag="ot")
                    nj = 4 * bi + 4
                    for bj in range(nj):
                        j_sl = slice(bj * P, (bj + 1) * P)
                        st = pst.tile([P, 1024], F32, tag="st")
                        nc.tensor.matmul(
                            st[:, 0:512], kt[0:64, j_sl], qt[0:64, i_sl],
                            start=True, stop=True, tile_position=(0, 0),
                        )
                        nc.tensor.matmul(
                            st[:, 512:1024], kt[64:128, j_sl], qt[64:128, i_sl],
                            start=True, stop=True, tile_position=(64, 0),
                        )
                        pt = ppool.tile([P, 1024], BF16, tag="pt")
                        nc.scalar.activation(pt, st, EXP, scale=SCALE)
                        if bj >= 4 * bi:
                            r = bj - 4 * bi
                            nc.vector.tensor_tensor(
                                pt[:, 0:512], pt[:, 0:512], maskt[r], MUL
                            )
                            nc.vector.tensor_tensor(
                                pt[:, 512:1024], pt[:, 512:1024], maskt[r], MUL
                            )
                        nc.tensor.matmul(
                            oA, vaug[2 * p][:, bj, :], pt[:, 0:512],
                            start=(bj == 0), stop=(bj == nj - 1),
                        )
                        nc.tensor.matmul(
                            oB, vaug[2 * p + 1][:, bj, :], pt[:, 512:1024],
                            start=(bj == 0), stop=(bj == nj - 1),
                        )
                    # Short psum-release tail: copy unnormalized oT + l rows
                    # out so the next block's PE matmuls aren't gated on the
                    # recip chain.
                    rc = sbm.tile([65, 512], F32, tag="rc")
                    nc.vector.memset(rc, 1.0)
                    nc.vector.tensor_copy(rc[0:1, :], oA[64:65, :])
                    nc.vector.tensor_copy(rc[64:65, :], oB[64:65, :])
                    nc.vector.tensor_copy(attT[p][0:64, i_sl], oA[0:64, :])
                    nc.vector.tensor_copy(attT[p][64:128, i_sl], oB[0:64, :])
                    # off-path: recip, hi/lo split, broadcast, normalize
                    nc.vector.reciprocal(rc, rc)
                    hi = sbm.tile([65, 512], BF16, tag="hi")
                    lof = sbm.tile([65, 512], F32, tag="lof")
                    lo = sbm.tile([65, 512], BF16, tag="lo")
                    nc.vector.tensor_copy(hi, rc)
                    nc.vector.tensor_tensor(lof, rc, hi, SUB)
                    nc.vector.tensor_copy(lo, lof)
                    bcp = pmisc.tile([P, 512], F32, tag="m")
                    nc.tensor.matmul(bcp, ind_sb, hi, start=True, stop=False)
                    nc.tensor.matmul(bcp, ind_sb, lo, start=False, stop=True)
                    bcs = sbm.tile([P, 512], F32, tag="bcs")
                    nc.vector.tensor_copy(bcs, bcp)
                    nc.vector.tensor_tensor(
                        attT[p][0:64, i_sl], attT[p][0:64, i_sl],
                        bcs[0:64, :], MUL
                    )
                    nc.vector.tensor_tensor(
                        attT[p][64:128, i_sl], attT[p][64:128, i_sl],
                        bcs[64:128, :], MUL
                    )

        # ---------------- output projection ----------------
        with tc.tile_pool(name="wop", bufs=1) as wop:
            wo = []
            for c in range(CC):
                w3 = wop.tile([P, C], BF16, tag=f"wo{c}")
                nc.sync.dma_start(w3, wo_d[c * P : (c + 1) * P, :])
                wo.append(w3)
            for t in range(NT128):
                y = ysb.tile([P, C], F32, tag="y")
                for oh in range(2):
                    py = pmisc.tile([P, 512], F32, tag="m")
                    for p in range(NP):
                        nc.tensor.matmul(
                            py,
                            attT[p][:, t * P : (t + 1) * P],
                            wo[p][:, oh * 512 : (oh + 1) * 512],
                            start=(p == 0),
                            stop=(p == NP - 1),
                        )
                    nc.vector.tensor_copy(y[:, oh * 512 : (oh + 1) * 512], py)
                nc.sync.dma_start(out_d[t * P : (t + 1) * P, :], y)

    nc.compile()
    return nc


_NC_CACHE = {}


def _get_nc(W_qkv, W_out):
    key = hashlib.sha1(W_qkv.tobytes() + W_out.tobytes()).hexdigest()
    nc = _NC_CACHE.get(key)
    if nc is None:
        nc = _build(*_prep_weights(W_qkv, W_out))
        _NC_CACHE[key] = nc
    return nc


def kernel(x, W_qkv, W_out):
    global LAST_RESULTS
    x = np.asarray(x, dtype=np.float32)
    W_qkv = np.asarray(W_qkv, dtype=np.float32)
    W_out = np.asarray(W_out, dtype=np.float32)
    B = x.shape[0]

    nc = _get_nc(W_qkv, W_out)
    in_maps = [
        {"xT": np.ascontiguousarray(x[b].T).astype(BF)} for b in range(B)
    ]
    res = run_bass_kernel_spmd(nc, in_maps, core_ids=list(range(B)))
    LAST_RESULTS = res

    y = np.empty((B, T, C), np.float32)
    for b in range(B):
        y[b] = res.results[b]["out"]
    return y
